# revision 1
# baseline (speedup 1.0000x reference)
"""Trainium2 Bass kernel for nn_Conv2d_14147622273082.

Conv2d 3x3, stride 1, pad 1: x [8, 320, 64, 64] f32, hf8-coded weights
w_bits [320, 320, 3, 3] i32 (codes 0..255), bias codes b_bits [320] i32.
out = conv2d(x, hf8_decode(w_bits)) + hf8_decode(b_bits).

Strategy: data-parallel over batch (1 image per NeuronCore, 8 cores).
Weights replicated; hf8 decode on-device via a bit trick:
hf8(1-4-3, bias 14) == bitcast_f32(sign<<31 | code7<<20) * 2^113
(exact, incl. subnormals). The conv is 9 shifted [Cin,Cout] x [Cin,pix]
fp16 matmuls accumulated in PSUM over a zero-padded fp16 input image.

Cin=320 splits into K-chunks (128, 128, 64). The 64-wide tail would waste
half the PE array, so kernel positions are packed in pairs: partitions
0:64 hold the tail channels, partitions 64:128 hold the same channels
with the padded image pre-shifted by the delta between the two positions
(flat +1 == next column; flat +66 == (row+1, col-2) in the 68-wide pad),
so one K=128 matmul computes two positions at once. 9 positions -> 4
pairs + 1 solo: 23 instead of 27 accumulating matmuls per PSUM tile.
"""

import numpy as np

import concourse.bass as bass
import concourse.tile as tile
from concourse import bacc, mybir
from concourse.bass_utils import run_bass_kernel_spmd

B, CIN, COUT, H, W = 8, 320, 320, 64, 64
PIX = H * W  # 4096
P = 128
CO_CHUNKS = [(0, 128), (128, 256), (256, 320)]
N_TILE = 512  # pixels per psum tile = 8 rows of 64
ROWS_PER_TILE = N_TILE // W  # 8
N_PIX_TILES = PIX // N_TILE  # 8
# padded image: rows 0..65 (top/bottom zero), cols: 2 left / 2 right zero
HP, WP = H + 2, W + 4  # 66 x 68 (even left pad keeps fp16 writes 4B-aligned)

# tail position pairing: (pos_a, pos_b) packed on partitions (0:64, 64:128).
# delta = flat_offset(b) - flat_offset(a) in the padded [66,68] layout.
# pairs with delta 1 share the "+1 shifted" upper image (xp2 upper half);
# the delta-66 pair gets its own tile (XB).
TAIL_PAIRS = [(0, 1), (2, 3), (4, 5), (6, 7)]  # pos = kh*3+kw
TAIL_SOLO = 8

F16 = mybir.dt.float16
F32 = mybir.dt.float32
I32 = mybir.dt.int32
HF8_SCALE = float(2.0**113)


def _decode_hf8(nc, pool, codes_ap, out_ap, nparts, free, tag, after=None):
    """out = hf8_decode(codes) = bitcast_f32(sign<<31 | code7<<20) * 2^113.

    Returns the last instruction. If `after` is given, the stage's first ops
    get no-sync ordering edges onto it so the Tile scheduler cannot hoist
    this stage ahead of earlier work on the engine (its compile-time DMA
    timing model underestimates HBM contention, which otherwise causes
    head-of-line stalls).
    """
    from concourse.tile_rust import add_dep_helper

    t1 = pool.tile([P, free], I32, tag=f"{tag}_t1", name=f"{tag}_t1")
    t2 = pool.tile([P, free], I32, tag=f"{tag}_t2", name=f"{tag}_t2")
    i1 = nc.vector.tensor_scalar(
        t1[:nparts], codes_ap, 0x80, 24,
        mybir.AluOpType.bitwise_and, mybir.AluOpType.logical_shift_left,
    )
    i2 = nc.vector.tensor_scalar(
        t2[:nparts], codes_ap, 0x7F, 20,
        mybir.AluOpType.bitwise_and, mybir.AluOpType.logical_shift_left,
    )
    if after is not None:
        add_dep_helper(i1.ins, after.ins, sync=False, reason="decode stage order")
        add_dep_helper(i2.ins, after.ins, sync=False, reason="decode stage order")
    nc.vector.tensor_tensor(
        t1[:nparts], t1[:nparts], t2[:nparts], mybir.AluOpType.bitwise_or
    )
    return nc.vector.tensor_scalar_mul(out_ap, t1[:nparts].bitcast(F32), HF8_SCALE)


def _pad_borders(nc, xt, col_lo, col_hi, parts=slice(0, P), rows=(0, HP - 1)):
    """Zero the pad borders around an interior written at cols [col_lo, col_hi)."""
    nc.vector.memset(xt[parts, rows[0] : rows[0] + 1, :], 0.0)
    nc.vector.memset(xt[parts, rows[1] : rows[1] + 1, :], 0.0)
    if col_lo > 0:
        nc.vector.memset(xt[parts, rows[0] + 1 : rows[1], 0:col_lo], 0.0)
    if col_hi < WP:
        nc.vector.memset(xt[parts, rows[0] + 1 : rows[1], col_hi:WP], 0.0)


def build():
    from concourse.tile_rust import add_dep_helper

    nc = bacc.Bacc(
        "TRN2", target_bir_lowering=False, debug=False, enable_partition_id=False
    )
    x_d = nc.dram_tensor("x", [CIN, PIX], F32, kind="ExternalInput")
    w_d = nc.dram_tensor("w9", [CIN, 9, COUT], I32, kind="ExternalInput")
    b_d = nc.dram_tensor("b", [3 * P, 1], I32, kind="ExternalInput")
    out_d = nc.dram_tensor("out", [COUT, PIX], F32, kind="ExternalOutput")

    with tile.TileContext(nc) as tc:
        with (
            tc.tile_pool(name="persist", bufs=1) as persist,
            tc.tile_pool(name="stage", bufs=1) as stage,
            tc.tile_pool(name="outsb", bufs=4) as outsb,
            tc.tile_pool(name="psum", bufs=1, space="PSUM") as psum_pool,
        ):
            # All input DMAs ride the sync queue, which processes them in
            # issue order: earliest-deadline first. hf8 decode runs on DVE,
            # pad casts on the Scalar engine, PSUM epilogue on Scalar.
            # ---- SBUF tiles ----
            wraw = [
                stage.tile([P, 9, COUT], I32, tag=f"wraw{c}", name=f"wraw{c}")
                for c in range(2)
            ]
            wt = [
                persist.tile([P, 9, COUT], F16, tag=f"wl{c}", name=f"wl{c}")
                for c in range(2)
            ]
            xs = [
                stage.tile([P, H, W], F32, tag=f"xstage{c}", name=f"xstage{c}")
                for c in range(2)
            ]
            xt = [
                persist.tile([P, HP, WP], F16, tag=f"xpad{c}", name=f"xpad{c}")
                for c in range(2)
            ]
            wraw2 = stage.tile([P, 5, COUT], I32, tag="wraw2", name="wraw2")
            wpair = persist.tile([P, 5, COUT], F16, tag="wpair", name="wpair")
            xs2 = stage.tile([P, H, W], F32, tag="xstage2", name="xstage2")
            xp2 = persist.tile([P, HP, WP], F16, tag="xpad2", name="xpad2")
            xb2 = persist.tile([P, HP, WP], F16, tag="xpadb", name="xpadb")
            wl = wt
            xp = xt

            # ---- input DMAs, deadline order, one in-order queue ----
            wfl = [w.rearrange("p a b -> p (a b)") for w in wraw]
            half = 5 * COUT
            nc.sync.dma_start(wfl[0][:, :half], w_d[0:P, :5])
            nc.sync.dma_start(
                xs[0][:, : H // 2],
                x_d[0:P, : PIX // 2].rearrange("p (h w) -> p h w", h=H // 2),
            )
            nc.sync.dma_start(wfl[0][:, half:], w_d[0:P, 5:])
            nc.sync.dma_start(
                xs[0][:, H // 2 :],
                x_d[0:P, PIX // 2 :].rearrange("p (h w) -> p h w", h=H // 2),
            )
            nc.sync.dma_start(wraw[1][:], w_d[P : 2 * P])
            nc.sync.dma_start(
                xs[1][:], x_d[P : 2 * P].rearrange("p (h w) -> p h w", h=H)
            )
            cs, ce = 256, 320
            nc.sync.dma_start(wraw2[0:64, 0:5], w_d[cs:ce, 0:9:2])
            nc.sync.dma_start(wraw2[64:128, 0:4], w_d[cs:ce, 1:9:2])
            nc.sync.dma_start(
                xs2[0:64], x_d[cs:ce].rearrange("p (h w) -> p h w", h=H)
            )
            nc.sync.dma_start(
                xs2[64:128], x_d[cs:ce].rearrange("p (h w) -> p h w", h=H)
            )
            braw = stage.tile([P, 3], I32, tag="braw", name="braw")
            nc.sync.dma_start(
                braw[:], b_d.rearrange("(a p) one -> p (a one)", p=P)
            )

            # ---- PE warmup: keep TensorE busy (HAM at 8/8) through the
            # prologue so the real stream starts at 2.4 GHz ----
            wsrc = stage.tile([P, P], F16, tag="wsrc", name="wsrc")
            nc.vector.memset(wsrc[:], 0.0)
            warm_ps = psum_pool.tile([P, N_TILE], F32, tag="acc0", name="warm_ps")
            for _ in range(150):
                nc.tensor.matmul(
                    warm_ps[:, 0:P], wsrc[:], wsrc[:], start=True, stop=True
                )

            # ---- borders (DVE, no data deps: fills the DMA wait) ----
            for c in range(2):
                _pad_borders(nc, xt[c], 2, W + 2)
            _pad_borders(nc, xp2, 2, W + 2, parts=slice(0, 64))
            _pad_borders(nc, xp2, 1, W + 1, parts=slice(64, P))
            _pad_borders(nc, xb2, 2, W + 2, parts=slice(0, 64))
            nc.vector.memset(xb2[64:128, H : HP, :], 0.0)
            nc.vector.memset(xb2[64:128, 0:H, 0:4], 0.0)
            nc.vector.memset(wraw2[64:128, 4], 0)

            # ---- Scalar-engine casts (warm the Copy table first), chained
            # in deadline order so the static schedule matches reality ----
            warm = stage.tile([P, 1], F32, tag="warm", name="warm")
            nc.vector.memset(warm[:], 0.0)
            a0 = nc.scalar.copy(warm[:], warm[:])
            a1 = nc.scalar.copy(
                xt[0][:, 1 : H // 2 + 1, 2 : W + 2], xs[0][:, : H // 2]
            )
            a2 = nc.scalar.copy(
                xt[0][:, H // 2 + 1 : H + 1, 2 : W + 2], xs[0][:, H // 2 :]
            )
            a3 = nc.scalar.copy(xt[1][:, 1 : H + 1, 2 : W + 2], xs[1][:])
            a4 = nc.scalar.copy(xp2[0:64, 1 : H + 1, 2 : W + 2], xs2[0:64])
            a5 = nc.scalar.copy(xp2[64:128, 1 : H + 1, 1 : W + 1], xs2[64:128])
            a6 = nc.scalar.copy(xb2[64:128, 0:H, 4:WP], xs2[64:128])
            prev = a0
            for a in (a1, a2, a3, a4, a5, a6):
                add_dep_helper(a.ins, prev.ins, sync=False, reason="cast order")
                prev = a

            # ---- hf8 decode on DVE, stage-chained in deadline order ----
            d1 = _decode_hf8(
                nc, stage, wfl[0][:, :half],
                wt[0].rearrange("p a b -> p (a b)")[:, :half], P, half, "wdec",
            )
            d2 = _decode_hf8(
                nc, stage, wfl[0][:, half:],
                wt[0].rearrange("p a b -> p (a b)")[:, half:],
                P, 9 * COUT - half, "wdec", after=d1,
            )
            d3 = _decode_hf8(
                nc, stage, wfl[1],
                wt[1].rearrange("p a b -> p (a b)"), P, 9 * COUT, "wdec", after=d2,
            )
            d4 = _decode_hf8(
                nc, stage,
                wraw2.rearrange("p a b -> p (a b)"),
                wpair.rearrange("p a b -> p (a b)"), P, 5 * COUT, "wdec2", after=d3,
            )
            # xb2 lower = same padded image as xp2 lower (same partitions)
            cpy = nc.vector.tensor_copy(
                xb2[0:64, 1 : H + 1, 2 : W + 2], xp2[0:64, 1 : H + 1, 2 : W + 2]
            )
            add_dep_helper(cpy.ins, d4.ins, sync=False, reason="tail copy order")
            bias = []
            prev = None
            for mi, (ms, me) in enumerate(CO_CHUNKS):
                pm = me - ms
                bf = persist.tile([P, 1], F32, tag=f"bias{mi}", name=f"bias{mi}")
                prev = _decode_hf8(
                    nc, stage, braw[:pm, mi : mi + 1], bf[:pm], pm, 1, "bdec",
                    after=prev if prev is not None else d4,
                )
                bias.append(bf)

            # ---- matmuls: out[co, pix] += w[ci,co].T @ x_shift[ci, pix] ----
            n_acc = 2 * 9 + len(TAIL_PAIRS) + 1  # 23 per psum tile
            for mi, (ms, me) in enumerate(CO_CHUNKS):
                pm = me - ms
                acc = [
                    psum_pool.tile(
                        [P, N_TILE], F32, tag=f"acc{t}", name=f"acc_{mi}_{t}"
                    )
                    for t in range(N_PIX_TILES)
                ]
                acc_k = [0] * N_PIX_TILES

                def mm(lhsT, src, kh, kw, t, pm=pm, acc=acc, acc_k=acc_k):
                    h0 = t * ROWS_PER_TILE
                    rhs = src[
                        : lhsT.shape[0],
                        h0 + kh : h0 + kh + ROWS_PER_TILE,
                        kw + 1 : kw + 1 + W,
                    ]
                    nc.tensor.matmul(
                        acc[t][:pm], lhsT, rhs,
                        start=(acc_k[t] == 0), stop=(acc_k[t] == n_acc - 1),
                    )
                    acc_k[t] += 1

                # For the very first co chunk, order chunk-0 work as
                # (weight half x image half) passes: the first 20 matmuls
                # need only the first 5 decoded positions and the first half
                # of the chunk-0 image.
                def tail_mms(t_range, pm=pm):
                    for j, (pa, pb) in enumerate(TAIL_PAIRS):
                        kh, kw = pa // 3, pa % 3
                        src = xb2 if (pa, pb) == (2, 3) else xp2
                        for t in t_range:
                            mm(wpair[:, j, ms:me], src, kh, kw, t)
                    for t in t_range:
                        mm(wpair[0:64, 4, ms:me], xp2, 2, 2, t)

                def epilogue(t, pm=pm, ms=ms, mi=mi):
                    osb = outsb.tile([P, N_TILE], F32, tag="osb", name="osb")
                    nc.scalar.activation(
                        osb[:pm], acc[t][:pm],
                        mybir.ActivationFunctionType.Identity,
                        bias=bias[mi][:pm], scale=1.0,
                    )
                    nc.sync.dma_start(
                        out_d[ms : ms + pm, t * N_TILE : (t + 1) * N_TILE], osb[:pm]
                    )

                if mi < 2:
                    if mi == 0:
                        c0_passes = [
                            (range(0, 5), range(0, 4)),
                            (range(5, 9), range(0, 4)),
                            (range(0, 5), range(4, 8)),
                            (range(5, 9), range(4, 8)),
                        ]
                    else:
                        c0_passes = [(range(9), range(N_PIX_TILES))]
                    for ci in range(2):
                        passes = (
                            c0_passes if ci == 0 else [(range(9), range(N_PIX_TILES))]
                        )
                        for pos_range, t_range in passes:
                            for pos in pos_range:
                                lhsT = wl[ci][:, pos, ms:me]
                                for t in t_range:
                                    mm(lhsT, xp[ci], pos // 3, pos % 3, t)
                    tail_mms(range(N_PIX_TILES))
                    assert all(k == n_acc for k in acc_k)
                    for t in range(N_PIX_TILES):
                        epilogue(t)
                else:
                    # last co chunk tile-by-tile: each PSUM tile finishes its
                    # 23 accumulations early so the Identity+bias epilogue
                    # overlaps the remaining stream instead of trailing it
                    for t in range(N_PIX_TILES):
                        for ci in range(2):
                            for pos in range(9):
                                mm(wl[ci][:, pos, ms:me], xp[ci], pos // 3, pos % 3, t)
                        tail_mms([t])
                        epilogue(t)
                    assert all(k == n_acc for k in acc_k)

    nc.compile()
    return nc


_NC_CACHE = None


def _get_nc():
    global _NC_CACHE
    if _NC_CACHE is None:
        _NC_CACHE = build()
    return _NC_CACHE


def _prep_in_maps(x, w_bits, b_bits):
    # w_bits [co, ci, kh, kw] -> [ci, kh*3+kw, co] (host relayout only)
    w9 = np.ascontiguousarray(
        w_bits.astype(np.int32).transpose(1, 2, 3, 0).reshape(CIN, 9, COUT)
    )
    b2 = np.zeros((3 * 128, 1), np.int32)
    b2[:COUT, 0] = b_bits.astype(np.int32).reshape(COUT)
    return [
        {
            "x": np.ascontiguousarray(x[i].reshape(CIN, PIX).astype(np.float32)),
            "w9": w9,
            "b": b2,
        }
        for i in range(B)
    ]


def kernel(x, w_bits, b_bits):
    nc = _get_nc()
    in_maps = _prep_in_maps(x, w_bits, b_bits)
    res = run_bass_kernel_spmd(nc, in_maps, core_ids=list(range(B)), trace=False)
    return np.stack(
        [res.results[i]["out"].reshape(COUT, H, W) for i in range(B)]
    ).astype(np.float32)


if __name__ == "__main__":
    rng = np.random.default_rng(0)
    x = rng.standard_normal((B, CIN, H, W)).astype(np.float32)
    w_bits = rng.integers(0, 256, (COUT, CIN, 3, 3)).astype(np.int32)
    b_bits = rng.integers(0, 256, (COUT,)).astype(np.int32)
    out = kernel(x, w_bits, b_bits)
    print("out", out.shape, out.dtype, float(np.abs(out).mean()))



# revision 6
# speedup vs baseline: 1.0088x; 1.0088x over previous
"""Trainium2 Bass kernel for nn_Conv2d_14147622273082.

Conv2d 3x3, stride 1, pad 1: x [8, 320, 64, 64] f32, hf8-coded weights
w_bits [320, 320, 3, 3] i32 (codes 0..255), bias codes b_bits [320] i32.
out = conv2d(x, hf8_decode(w_bits)) + hf8_decode(b_bits).

Strategy: data-parallel over batch (1 image per NeuronCore, 8 cores).
Weights replicated; hf8 decode on-device via a bit trick:
hf8(1-4-3, bias 14) == bitcast_f32(sign<<31 | code7<<20) * 2^113
(exact, incl. subnormals). The conv is 9 shifted [Cin,Cout] x [Cin,pix]
fp16 matmuls accumulated in PSUM over a zero-padded fp16 input image.

Cin=320 splits into K-chunks (128, 128, 64). The 64-wide tail would waste
half the PE array, so kernel positions are packed in pairs: partitions
0:64 hold the tail channels, partitions 64:128 hold the same channels
with the padded image pre-shifted by the delta between the two positions
(flat +1 == next column; flat +66 == (row+1, col-2) in the 68-wide pad),
so one K=128 matmul computes two positions at once. 9 positions -> 4
pairs + 1 solo: 23 instead of 27 accumulating matmuls per PSUM tile.
"""

import numpy as np

import concourse.bass as bass
import concourse.tile as tile
from concourse import bacc, mybir
from concourse.bass_utils import run_bass_kernel_spmd

B, CIN, COUT, H, W = 8, 320, 320, 64, 64
PIX = H * W  # 4096
P = 128
CO_CHUNKS = [(0, 128), (128, 256), (256, 320)]
N_TILE = 512  # pixels per psum tile = 8 rows of 64
ROWS_PER_TILE = N_TILE // W  # 8
N_PIX_TILES = PIX // N_TILE  # 8
# padded image: rows 0..65 (top/bottom zero), cols: 2 left / 2 right zero
HP, WP = H + 2, W + 4  # 66 x 68 (even left pad keeps fp16 writes 4B-aligned)

# tail position pairing: (pos_a, pos_b) packed on partitions (0:64, 64:128).
# delta = flat_offset(b) - flat_offset(a) in the padded [66,68] layout.
# pairs with delta 1 share the "+1 shifted" upper image (xp2 upper half);
# the delta-66 pair gets its own tile (XB).
TAIL_PAIRS = [(0, 1), (2, 3), (4, 5), (6, 7)]  # pos = kh*3+kw
TAIL_SOLO = 8

F16 = mybir.dt.float16
F32 = mybir.dt.float32
I32 = mybir.dt.int32
U8 = mybir.dt.uint8
HF8_SCALE = float(2.0**113)
N_WARMUP = 55


def _decode_hf8(nc, pool, codes_ap, out_ap, nparts, free, tag, after=None):
    """out = hf8_decode(codes) = bitcast_f32(sign<<31 | code7<<20) * 2^113.

    Returns the last instruction. If `after` is given, the stage's first ops
    get no-sync ordering edges onto it so the Tile scheduler cannot hoist
    this stage ahead of earlier work on the engine (its compile-time DMA
    timing model underestimates HBM contention, which otherwise causes
    head-of-line stalls).
    """
    from concourse.tile_rust import add_dep_helper

    t0 = pool.tile([P, free], I32, tag=f"{tag}_t0", name=f"{tag}_t0")
    t1 = pool.tile([P, free], I32, tag=f"{tag}_t1", name=f"{tag}_t1")
    t2 = pool.tile([P, free], I32, tag=f"{tag}_t2", name=f"{tag}_t2")
    # widen u8 codes -> i32 (DVE converts via f32; 0..255 exact)
    i0 = nc.vector.tensor_copy(t0[:nparts], codes_ap)
    i1 = nc.vector.tensor_scalar(
        t1[:nparts], t0[:nparts], 0x80, 24,
        mybir.AluOpType.bitwise_and, mybir.AluOpType.logical_shift_left,
    )
    i2 = nc.vector.tensor_scalar(
        t2[:nparts], t0[:nparts], 0x7F, 20,
        mybir.AluOpType.bitwise_and, mybir.AluOpType.logical_shift_left,
    )
    if after is not None:
        add_dep_helper(i0.ins, after.ins, sync=False, reason="decode stage order")
    nc.vector.tensor_tensor(
        t1[:nparts], t1[:nparts], t2[:nparts], mybir.AluOpType.bitwise_or
    )
    return nc.vector.tensor_scalar_mul(out_ap, t1[:nparts].bitcast(F32), HF8_SCALE)


def _pad_borders(nc, xt, col_lo, col_hi, parts=slice(0, P), rows=(0, HP - 1)):
    """Zero the pad borders around an interior written at cols [col_lo, col_hi)."""
    nc.vector.memset(xt[parts, rows[0] : rows[0] + 1, :], 0.0)
    nc.vector.memset(xt[parts, rows[1] : rows[1] + 1, :], 0.0)
    if col_lo > 0:
        nc.vector.memset(xt[parts, rows[0] + 1 : rows[1], 0:col_lo], 0.0)
    if col_hi < WP:
        nc.vector.memset(xt[parts, rows[0] + 1 : rows[1], col_hi:WP], 0.0)


def build():
    from concourse.tile_rust import add_dep_helper

    nc = bacc.Bacc(
        "TRN2", target_bir_lowering=False, debug=False, enable_partition_id=False
    )
    x_d = nc.dram_tensor("x", [CIN, PIX], F32, kind="ExternalInput")
    w_d = nc.dram_tensor("w9", [CIN, 9, COUT], U8, kind="ExternalInput")
    b_d = nc.dram_tensor("b", [3 * P, 1], U8, kind="ExternalInput")
    out_d = nc.dram_tensor("out", [COUT, PIX], F32, kind="ExternalOutput")

    with tile.TileContext(nc) as tc:
        with (
            tc.tile_pool(name="persist", bufs=1) as persist,
            tc.tile_pool(name="stage", bufs=1) as stage,
            tc.tile_pool(name="outsb", bufs=4) as outsb,
            tc.tile_pool(name="psum", bufs=1, space="PSUM") as psum_pool,
        ):
            # All input DMAs ride the sync queue, which processes them in
            # issue order: earliest-deadline first. hf8 decode runs on DVE,
            # pad casts on the Scalar engine, PSUM epilogue on Scalar.
            # ---- SBUF tiles ----
            wraw = [
                stage.tile([P, 9, COUT], U8, tag=f"wraw{c}", name=f"wraw{c}")
                for c in range(2)
            ]
            wt = [
                persist.tile([P, 9, COUT], F16, tag=f"wl{c}", name=f"wl{c}")
                for c in range(2)
            ]
            xs = [
                stage.tile([P, H, W], F32, tag=f"xstage{c}", name=f"xstage{c}")
                for c in range(2)
            ]
            xt = [
                persist.tile([P, HP, WP], F16, tag=f"xpad{c}", name=f"xpad{c}")
                for c in range(2)
            ]
            wraw2 = stage.tile([P, 5, COUT], U8, tag="wraw2", name="wraw2")
            wpair = persist.tile([P, 5, COUT], F16, tag="wpair", name="wpair")
            xs2 = stage.tile([P, H, W], F32, tag="xstage2", name="xstage2")
            xp2 = persist.tile([P, HP, WP], F16, tag="xpad2", name="xpad2")
            xb2 = persist.tile([P, HP, WP], F16, tag="xpadb", name="xpadb")
            wl = wt
            xp = xt

            # ---- input DMAs, deadline order, one in-order queue ----
            wfl = [w.rearrange("p a b -> p (a b)") for w in wraw]
            half = 5 * COUT
            nc.sync.dma_start(wfl[0][:, :half], w_d[0:P, :5])
            nc.sync.dma_start(
                xs[0][:, : H // 2],
                x_d[0:P, : PIX // 2].rearrange("p (h w) -> p h w", h=H // 2),
            )
            nc.sync.dma_start(wfl[0][:, half:], w_d[0:P, 5:])
            nc.sync.dma_start(
                xs[0][:, H // 2 :],
                x_d[0:P, PIX // 2 :].rearrange("p (h w) -> p h w", h=H // 2),
            )
            nc.sync.dma_start(wraw[1][:], w_d[P : 2 * P])
            nc.sync.dma_start(
                xs[1][:], x_d[P : 2 * P].rearrange("p (h w) -> p h w", h=H)
            )
            cs, ce = 256, 320
            nc.sync.dma_start(wraw2[0:64, 0:5], w_d[cs:ce, 0:9:2])
            nc.sync.dma_start(wraw2[64:128, 0:4], w_d[cs:ce, 1:9:2])
            nc.sync.dma_start(
                xs2[0:64], x_d[cs:ce].rearrange("p (h w) -> p h w", h=H)
            )
            nc.sync.dma_start(
                xs2[64:128], x_d[cs:ce].rearrange("p (h w) -> p h w", h=H)
            )
            braw = stage.tile([P, 3], U8, tag="braw", name="braw")
            nc.sync.dma_start(
                braw[:], b_d.rearrange("(a p) one -> p (a one)", p=P)
            )

            # ---- PE warmup: keep TensorE busy (HAM at 8/8) through the
            # prologue so the real stream starts at 2.4 GHz ----
            wsrc = stage.tile([P, P], F16, tag="wsrc", name="wsrc")
            nc.vector.memset(wsrc[:], 0.0)
            warm_ps = psum_pool.tile([P, N_TILE], F32, tag="acc0", name="warm_ps")
            for _ in range(N_WARMUP):
                nc.tensor.matmul(
                    warm_ps[:, 0:P], wsrc[:], wsrc[:], start=True, stop=True
                )

            # ---- borders (DVE, no data deps: fills the DMA wait) ----
            for c in range(2):
                _pad_borders(nc, xt[c], 2, W + 2)
            _pad_borders(nc, xp2, 2, W + 2, parts=slice(0, 64))
            _pad_borders(nc, xp2, 1, W + 1, parts=slice(64, P))
            _pad_borders(nc, xb2, 2, W + 2, parts=slice(0, 64))
            nc.vector.memset(xb2[64:128, H : HP, :], 0.0)
            nc.vector.memset(xb2[64:128, 0:H, 0:4], 0.0)
            nc.vector.memset(wraw2[64:128, 4], 0)

            # ---- Scalar-engine casts (warm the Copy table first), chained
            # in deadline order so the static schedule matches reality ----
            warm = stage.tile([P, 1], F32, tag="warm", name="warm")
            nc.vector.memset(warm[:], 0.0)
            a0 = nc.scalar.copy(warm[:], warm[:])
            a1 = nc.scalar.copy(
                xt[0][:, 1 : H // 2 + 1, 2 : W + 2], xs[0][:, : H // 2]
            )
            a2 = nc.scalar.copy(
                xt[0][:, H // 2 + 1 : H + 1, 2 : W + 2], xs[0][:, H // 2 :]
            )
            a3 = nc.scalar.copy(xt[1][:, 1 : H + 1, 2 : W + 2], xs[1][:])
            a4 = nc.scalar.copy(xp2[0:64, 1 : H + 1, 2 : W + 2], xs2[0:64])
            a5 = nc.scalar.copy(xp2[64:128, 1 : H + 1, 1 : W + 1], xs2[64:128])
            a6 = nc.scalar.copy(xb2[64:128, 0:H, 4:WP], xs2[64:128])
            prev = a0
            for a in (a1, a2, a3, a4, a5, a6):
                add_dep_helper(a.ins, prev.ins, sync=False, reason="cast order")
                prev = a

            # ---- hf8 decode on DVE, stage-chained in deadline order ----
            d1 = _decode_hf8(
                nc, stage, wfl[0][:, :half],
                wt[0].rearrange("p a b -> p (a b)")[:, :half], P, half, "wdec",
            )
            d2 = _decode_hf8(
                nc, stage, wfl[0][:, half:],
                wt[0].rearrange("p a b -> p (a b)")[:, half:],
                P, 9 * COUT - half, "wdec", after=d1,
            )
            d3 = _decode_hf8(
                nc, stage, wfl[1],
                wt[1].rearrange("p a b -> p (a b)"), P, 9 * COUT, "wdec", after=d2,
            )
            d4 = _decode_hf8(
                nc, stage,
                wraw2.rearrange("p a b -> p (a b)"),
                wpair.rearrange("p a b -> p (a b)"), P, 5 * COUT, "wdec2", after=d3,
            )
            # xb2 lower = same padded image as xp2 lower (same partitions)
            cpy = nc.vector.tensor_copy(
                xb2[0:64, 1 : H + 1, 2 : W + 2], xp2[0:64, 1 : H + 1, 2 : W + 2]
            )
            add_dep_helper(cpy.ins, d4.ins, sync=False, reason="tail copy order")
            bias = []
            prev = None
            for mi, (ms, me) in enumerate(CO_CHUNKS):
                pm = me - ms
                bf = persist.tile([P, 1], F32, tag=f"bias{mi}", name=f"bias{mi}")
                prev = _decode_hf8(
                    nc, stage, braw[:pm, mi : mi + 1], bf[:pm], pm, 1, "bdec",
                    after=prev if prev is not None else d4,
                )
                bias.append(bf)

            # ---- matmuls: out[co, pix] += w[ci,co].T @ x_shift[ci, pix] ----
            n_acc = 2 * 9 + len(TAIL_PAIRS) + 1  # 23 per psum tile
            for mi, (ms, me) in enumerate(CO_CHUNKS):
                pm = me - ms
                acc = [
                    psum_pool.tile(
                        [P, N_TILE], F32, tag=f"acc{t}", name=f"acc_{mi}_{t}"
                    )
                    for t in range(N_PIX_TILES)
                ]
                acc_k = [0] * N_PIX_TILES

                def mm(lhsT, src, kh, kw, t, pm=pm, acc=acc, acc_k=acc_k):
                    h0 = t * ROWS_PER_TILE
                    rhs = src[
                        : lhsT.shape[0],
                        h0 + kh : h0 + kh + ROWS_PER_TILE,
                        kw + 1 : kw + 1 + W,
                    ]
                    nc.tensor.matmul(
                        acc[t][:pm], lhsT, rhs,
                        start=(acc_k[t] == 0), stop=(acc_k[t] == n_acc - 1),
                    )
                    acc_k[t] += 1

                # For the very first co chunk, order chunk-0 work as
                # (weight half x image half) passes: the first 20 matmuls
                # need only the first 5 decoded positions and the first half
                # of the chunk-0 image.
                def tail_mms(t_range, pm=pm):
                    for j, (pa, pb) in enumerate(TAIL_PAIRS):
                        kh, kw = pa // 3, pa % 3
                        src = xb2 if (pa, pb) == (2, 3) else xp2
                        for t in t_range:
                            mm(wpair[:, j, ms:me], src, kh, kw, t)
                    for t in t_range:
                        mm(wpair[0:64, 4, ms:me], xp2, 2, 2, t)

                def epilogue(t, pm=pm, ms=ms, mi=mi):
                    osb = outsb.tile([P, N_TILE], F32, tag="osb", name="osb")
                    nc.scalar.activation(
                        osb[:pm], acc[t][:pm],
                        mybir.ActivationFunctionType.Identity,
                        bias=bias[mi][:pm], scale=1.0,
                    )
                    nc.sync.dma_start(
                        out_d[ms : ms + pm, t * N_TILE : (t + 1) * N_TILE], osb[:pm]
                    )

                if mi < 2:
                    if mi == 0:
                        c0_passes = [
                            (range(0, 5), range(0, 4)),
                            (range(5, 9), range(0, 4)),
                            (range(0, 5), range(4, 8)),
                            (range(5, 9), range(4, 8)),
                        ]
                    else:
                        c0_passes = [(range(9), range(N_PIX_TILES))]
                    for ci in range(2):
                        passes = (
                            c0_passes if ci == 0 else [(range(9), range(N_PIX_TILES))]
                        )
                        for pos_range, t_range in passes:
                            for pos in pos_range:
                                lhsT = wl[ci][:, pos, ms:me]
                                for t in t_range:
                                    mm(lhsT, xp[ci], pos // 3, pos % 3, t)
                    tail_mms(range(N_PIX_TILES))
                    assert all(k == n_acc for k in acc_k)
                    for t in range(N_PIX_TILES):
                        epilogue(t)
                else:
                    # last co chunk tile-by-tile: each PSUM tile finishes its
                    # 23 accumulations early so the Identity+bias epilogue
                    # overlaps the remaining stream instead of trailing it
                    for t in range(N_PIX_TILES):
                        for ci in range(2):
                            for pos in range(9):
                                mm(wl[ci][:, pos, ms:me], xp[ci], pos // 3, pos % 3, t)
                        tail_mms([t])
                        epilogue(t)
                    assert all(k == n_acc for k in acc_k)

    nc.compile()
    return nc


_NC_CACHE = None


def _get_nc():
    global _NC_CACHE
    if _NC_CACHE is None:
        _NC_CACHE = build()
    return _NC_CACHE


def _prep_in_maps(x, w_bits, b_bits):
    # w_bits [co, ci, kh, kw] -> [ci, kh*3+kw, co] u8 codes (host relayout only)
    w9 = np.ascontiguousarray(
        w_bits.astype(np.uint8).transpose(1, 2, 3, 0).reshape(CIN, 9, COUT)
    )
    b2 = np.zeros((3 * 128, 1), np.uint8)
    b2[:COUT, 0] = b_bits.astype(np.uint8).reshape(COUT)
    return [
        {
            "x": np.ascontiguousarray(x[i].reshape(CIN, PIX).astype(np.float32)),
            "w9": w9,
            "b": b2,
        }
        for i in range(B)
    ]


def kernel(x, w_bits, b_bits):
    nc = _get_nc()
    in_maps = _prep_in_maps(x, w_bits, b_bits)
    res = run_bass_kernel_spmd(nc, in_maps, core_ids=list(range(B)), trace=False)
    return np.stack(
        [res.results[i]["out"].reshape(COUT, H, W) for i in range(B)]
    ).astype(np.float32)


if __name__ == "__main__":
    rng = np.random.default_rng(0)
    x = rng.standard_normal((B, CIN, H, W)).astype(np.float32)
    w_bits = rng.integers(0, 256, (COUT, CIN, 3, 3)).astype(np.int32)
    b_bits = rng.integers(0, 256, (COUT,)).astype(np.int32)
    out = kernel(x, w_bits, b_bits)
    print("out", out.shape, out.dtype, float(np.abs(out).mean()))



# revision 8
# speedup vs baseline: 1.0131x; 1.0043x over previous
"""Trainium2 Bass kernel for nn_Conv2d_14147622273082.

Conv2d 3x3, stride 1, pad 1: x [8, 320, 64, 64] f32, hf8-coded weights
w_bits [320, 320, 3, 3] i32 (codes 0..255), bias codes b_bits [320] i32.
out = conv2d(x, hf8_decode(w_bits)) + hf8_decode(b_bits).

Strategy: data-parallel over batch (1 image per NeuronCore, 8 cores).
Weights replicated; hf8 decode on-device via a bit trick:
hf8(1-4-3, bias 14) == bitcast_f32(sign<<31 | code7<<20) * 2^113
(exact, incl. subnormals). The conv is 9 shifted [Cin,Cout] x [Cin,pix]
fp16 matmuls accumulated in PSUM over a zero-padded fp16 input image.

Cin=320 splits into K-chunks (128, 128, 64). The 64-wide tail would waste
half the PE array, so kernel positions are packed in pairs: partitions
0:64 hold the tail channels, partitions 64:128 hold the same channels
with the padded image pre-shifted by the delta between the two positions
(flat +1 == next column; flat +66 == (row+1, col-2) in the 68-wide pad),
so one K=128 matmul computes two positions at once. 9 positions -> 4
pairs + 1 solo: 23 instead of 27 accumulating matmuls per PSUM tile.
"""

import numpy as np

import concourse.bass as bass
import concourse.tile as tile
from concourse import bacc, mybir
from concourse.bass_utils import run_bass_kernel_spmd

B, CIN, COUT, H, W = 8, 320, 320, 64, 64
PIX = H * W  # 4096
P = 128
CO_CHUNKS = [(0, 128), (128, 256), (256, 320)]
N_TILE = 512  # pixels per psum tile = 8 rows of 64
ROWS_PER_TILE = N_TILE // W  # 8
N_PIX_TILES = PIX // N_TILE  # 8
# padded image: rows 0..65 (top/bottom zero), cols: 2 left / 2 right zero
HP, WP = H + 2, W + 4  # 66 x 68 (even left pad keeps fp16 writes 4B-aligned)

# tail position pairing: (pos_a, pos_b) packed on partitions (0:64, 64:128).
# delta = flat_offset(b) - flat_offset(a) in the padded [66,68] layout.
# pairs with delta 1 share the "+1 shifted" upper image (xp2 upper half);
# the delta-66 pair gets its own tile (XB).
TAIL_PAIRS = [(0, 1), (2, 3), (4, 5), (6, 7)]  # pos = kh*3+kw
TAIL_SOLO = 8

F16 = mybir.dt.float16
F32 = mybir.dt.float32
I32 = mybir.dt.int32
U8 = mybir.dt.uint8
HF8_SCALE = float(2.0**113)
N_WARMUP = 80


def _decode_hf8(nc, pool, codes_ap, out_ap, nparts, free, tag, after=None):
    """out = hf8_decode(codes) = bitcast_f32(sign<<31 | code7<<20) * 2^113.

    Returns the last instruction. If `after` is given, the stage's first ops
    get no-sync ordering edges onto it so the Tile scheduler cannot hoist
    this stage ahead of earlier work on the engine (its compile-time DMA
    timing model underestimates HBM contention, which otherwise causes
    head-of-line stalls).
    """
    from concourse.tile_rust import add_dep_helper

    t0 = pool.tile([P, free], I32, tag=f"{tag}_t0", name=f"{tag}_t0")
    t1 = pool.tile([P, free], I32, tag=f"{tag}_t1", name=f"{tag}_t1")
    t2 = pool.tile([P, free], I32, tag=f"{tag}_t2", name=f"{tag}_t2")
    # widen u8 codes -> i32 (DVE converts via f32; 0..255 exact)
    i0 = nc.vector.tensor_copy(t0[:nparts], codes_ap)
    i1 = nc.vector.tensor_scalar(
        t1[:nparts], t0[:nparts], 0x80, 24,
        mybir.AluOpType.bitwise_and, mybir.AluOpType.logical_shift_left,
    )
    i2 = nc.vector.tensor_scalar(
        t2[:nparts], t0[:nparts], 0x7F, 20,
        mybir.AluOpType.bitwise_and, mybir.AluOpType.logical_shift_left,
    )
    if after is not None:
        add_dep_helper(i0.ins, after.ins, sync=False, reason="decode stage order")
    nc.vector.tensor_tensor(
        t1[:nparts], t1[:nparts], t2[:nparts], mybir.AluOpType.bitwise_or
    )
    return nc.vector.tensor_scalar_mul(out_ap, t1[:nparts].bitcast(F32), HF8_SCALE)


def _pad_borders(nc, xt, col_lo, col_hi, parts=slice(0, P), rows=(0, HP - 1)):
    """Zero the pad borders around an interior written at cols [col_lo, col_hi)."""
    nc.vector.memset(xt[parts, rows[0] : rows[0] + 1, :], 0.0)
    nc.vector.memset(xt[parts, rows[1] : rows[1] + 1, :], 0.0)
    if col_lo > 0:
        nc.vector.memset(xt[parts, rows[0] + 1 : rows[1], 0:col_lo], 0.0)
    if col_hi < WP:
        nc.vector.memset(xt[parts, rows[0] + 1 : rows[1], col_hi:WP], 0.0)


def build():
    from concourse.tile_rust import add_dep_helper

    nc = bacc.Bacc(
        "TRN2", target_bir_lowering=False, debug=False, enable_partition_id=False
    )
    x_d = nc.dram_tensor("x", [CIN, PIX], F32, kind="ExternalInput")
    w_d = nc.dram_tensor("w9", [CIN, 9, COUT], U8, kind="ExternalInput")
    b_d = nc.dram_tensor("b", [3 * P, 1], U8, kind="ExternalInput")
    out_d = nc.dram_tensor("out", [COUT, PIX], F32, kind="ExternalOutput")

    with tile.TileContext(nc) as tc:
        with (
            tc.tile_pool(name="persist", bufs=1) as persist,
            tc.tile_pool(name="stage", bufs=1) as stage,
            tc.tile_pool(name="outsb", bufs=4) as outsb,
            tc.tile_pool(name="psum", bufs=1, space="PSUM") as psum_pool,
        ):
            # All input DMAs ride the sync queue, which processes them in
            # issue order: earliest-deadline first. hf8 decode runs on DVE,
            # pad casts on the Scalar engine, PSUM epilogue on Scalar.
            # ---- SBUF tiles ----
            wraw = [
                stage.tile([P, 9, COUT], U8, tag=f"wraw{c}", name=f"wraw{c}")
                for c in range(2)
            ]
            wt = [
                persist.tile([P, 9, COUT], F16, tag=f"wl{c}", name=f"wl{c}")
                for c in range(2)
            ]
            xs = [
                stage.tile([P, H, W], F32, tag=f"xstage{c}", name=f"xstage{c}")
                for c in range(2)
            ]
            xt = [
                persist.tile([P, HP, WP], F16, tag=f"xpad{c}", name=f"xpad{c}")
                for c in range(2)
            ]
            wraw2 = stage.tile([P, 5, COUT], U8, tag="wraw2", name="wraw2")
            wpair = persist.tile([P, 5, COUT], F16, tag="wpair", name="wpair")
            xs2 = stage.tile([P, H, W], F32, tag="xstage2", name="xstage2")
            xp2 = persist.tile([P, HP, WP], F16, tag="xpad2", name="xpad2")
            xb2 = persist.tile([P, HP, WP], F16, tag="xpadb", name="xpadb")
            wl = wt
            xp = xt

            # ---- input DMAs, deadline order, one in-order queue ----
            wfl = [w.rearrange("p a b -> p (a b)") for w in wraw]
            half = 5 * COUT
            nc.sync.dma_start(wfl[0][:, :half], w_d[0:P, :5])
            nc.sync.dma_start(
                xs[0][:, : H // 2],
                x_d[0:P, : PIX // 2].rearrange("p (h w) -> p h w", h=H // 2),
            )
            nc.sync.dma_start(wfl[0][:, half:], w_d[0:P, 5:])
            nc.sync.dma_start(
                xs[0][:, H // 2 :],
                x_d[0:P, PIX // 2 :].rearrange("p (h w) -> p h w", h=H // 2),
            )
            nc.sync.dma_start(wraw[1][:], w_d[P : 2 * P])
            nc.sync.dma_start(
                xs[1][:], x_d[P : 2 * P].rearrange("p (h w) -> p h w", h=H)
            )
            cs, ce = 256, 320
            nc.sync.dma_start(wraw2[0:64, 0:5], w_d[cs:ce, 0:9:2])
            nc.sync.dma_start(wraw2[64:128, 0:4], w_d[cs:ce, 1:9:2])
            nc.sync.dma_start(
                xs2[0:64], x_d[cs:ce].rearrange("p (h w) -> p h w", h=H)
            )
            nc.sync.dma_start(
                xs2[64:128], x_d[cs:ce].rearrange("p (h w) -> p h w", h=H)
            )
            braw = stage.tile([P, 3], U8, tag="braw", name="braw")
            nc.sync.dma_start(
                braw[:], b_d.rearrange("(a p) one -> p (a one)", p=P)
            )

            # ---- PE warmup: keep TensorE busy (HAM at 8/8) through the
            # prologue so the real stream starts at 2.4 GHz ----
            wsrc = stage.tile([P, P], F16, tag="wsrc", name="wsrc")
            nc.vector.memset(wsrc[:], 0.0)
            warm_ps = psum_pool.tile([P, N_TILE], F32, tag="acc0", name="warm_ps")
            for _ in range(N_WARMUP):
                nc.tensor.matmul(
                    warm_ps[:, 0:P], wsrc[:], wsrc[:], start=True, stop=True
                )

            # ---- borders (DVE, no data deps: fills the DMA wait) ----
            for c in range(2):
                _pad_borders(nc, xt[c], 2, W + 2)
            _pad_borders(nc, xp2, 2, W + 2, parts=slice(0, 64))
            _pad_borders(nc, xp2, 1, W + 1, parts=slice(64, P))
            _pad_borders(nc, xb2, 2, W + 2, parts=slice(0, 64))
            nc.vector.memset(xb2[64:128, H : HP, :], 0.0)
            nc.vector.memset(xb2[64:128, 0:H, 0:4], 0.0)
            nc.vector.memset(wraw2[64:128, 4], 0)

            # ---- Scalar-engine casts (warm the Copy table first), chained
            # in deadline order so the static schedule matches reality ----
            warm = stage.tile([P, 1], F32, tag="warm", name="warm")
            nc.vector.memset(warm[:], 0.0)
            a0 = nc.scalar.copy(warm[:], warm[:])
            a1 = nc.scalar.copy(
                xt[0][:, 1 : H // 2 + 1, 2 : W + 2], xs[0][:, : H // 2]
            )
            a2 = nc.scalar.copy(
                xt[0][:, H // 2 + 1 : H + 1, 2 : W + 2], xs[0][:, H // 2 :]
            )
            a3 = nc.scalar.copy(xt[1][:, 1 : H + 1, 2 : W + 2], xs[1][:])
            a4 = nc.scalar.copy(xp2[0:64, 1 : H + 1, 2 : W + 2], xs2[0:64])
            a5 = nc.scalar.copy(xp2[64:128, 1 : H + 1, 1 : W + 1], xs2[64:128])
            a6 = nc.scalar.copy(xb2[64:128, 0:H, 4:WP], xs2[64:128])
            prev = a0
            for a in (a1, a2, a3, a4, a5, a6):
                add_dep_helper(a.ins, prev.ins, sync=False, reason="cast order")
                prev = a

            # ---- hf8 decode on DVE, stage-chained in deadline order ----
            d1 = _decode_hf8(
                nc, stage, wfl[0][:, :half],
                wt[0].rearrange("p a b -> p (a b)")[:, :half], P, half, "wdec",
            )
            d2 = _decode_hf8(
                nc, stage, wfl[0][:, half:],
                wt[0].rearrange("p a b -> p (a b)")[:, half:],
                P, 9 * COUT - half, "wdec", after=d1,
            )
            d3 = _decode_hf8(
                nc, stage, wfl[1],
                wt[1].rearrange("p a b -> p (a b)"), P, 9 * COUT, "wdec", after=d2,
            )
            d4 = _decode_hf8(
                nc, stage,
                wraw2.rearrange("p a b -> p (a b)"),
                wpair.rearrange("p a b -> p (a b)"), P, 5 * COUT, "wdec2", after=d3,
            )
            # xb2 lower = same padded image as xp2 lower (same partitions)
            cpy = nc.vector.tensor_copy(
                xb2[0:64, 1 : H + 1, 2 : W + 2], xp2[0:64, 1 : H + 1, 2 : W + 2]
            )
            add_dep_helper(cpy.ins, d4.ins, sync=False, reason="tail copy order")
            bias = []
            prev = None
            for mi, (ms, me) in enumerate(CO_CHUNKS):
                pm = me - ms
                bf = persist.tile([P, 1], F32, tag=f"bias{mi}", name=f"bias{mi}")
                prev = _decode_hf8(
                    nc, stage, braw[:pm, mi : mi + 1], bf[:pm], pm, 1, "bdec",
                    after=prev if prev is not None else d4,
                )
                bias.append(bf)

            # ---- matmuls: out[co, pix] += w[ci,co].T @ x_shift[ci, pix] ----
            n_acc = 2 * 9 + len(TAIL_PAIRS) + 1  # 23 per psum tile
            for mi, (ms, me) in enumerate(CO_CHUNKS):
                pm = me - ms
                acc = [
                    psum_pool.tile(
                        [P, N_TILE], F32, tag=f"acc{t}", name=f"acc_{mi}_{t}"
                    )
                    for t in range(N_PIX_TILES)
                ]
                acc_k = [0] * N_PIX_TILES

                def mm(lhsT, src, kh, kw, t, pm=pm, acc=acc, acc_k=acc_k):
                    h0 = t * ROWS_PER_TILE
                    rhs = src[
                        : lhsT.shape[0],
                        h0 + kh : h0 + kh + ROWS_PER_TILE,
                        kw + 1 : kw + 1 + W,
                    ]
                    nc.tensor.matmul(
                        acc[t][:pm], lhsT, rhs,
                        start=(acc_k[t] == 0), stop=(acc_k[t] == n_acc - 1),
                    )
                    acc_k[t] += 1

                # For the very first co chunk, order chunk-0 work as
                # (weight half x image half) passes: the first 20 matmuls
                # need only the first 5 decoded positions and the first half
                # of the chunk-0 image.
                def tail_mms(t_range, pm=pm):
                    for j, (pa, pb) in enumerate(TAIL_PAIRS):
                        kh, kw = pa // 3, pa % 3
                        src = xb2 if (pa, pb) == (2, 3) else xp2
                        for t in t_range:
                            mm(wpair[:, j, ms:me], src, kh, kw, t)
                    for t in t_range:
                        mm(wpair[0:64, 4, ms:me], xp2, 2, 2, t)

                def epilogue(t, pm=pm, ms=ms, mi=mi):
                    osb = outsb.tile([P, N_TILE], F32, tag="osb", name="osb")
                    nc.scalar.activation(
                        osb[:pm], acc[t][:pm],
                        mybir.ActivationFunctionType.Identity,
                        bias=bias[mi][:pm], scale=1.0,
                    )
                    nc.sync.dma_start(
                        out_d[ms : ms + pm, t * N_TILE : (t + 1) * N_TILE], osb[:pm]
                    )

                if mi < 2:
                    if mi == 0:
                        # tiles 0-2 only need the top-half image cast (padded
                        # rows <= 25), so the stream can start before the
                        # bottom-half DMA lands
                        c0_passes = [
                            (range(0, 5), range(0, 3)),
                            (range(5, 9), range(0, 3)),
                            (range(0, 5), range(3, 8)),
                            (range(5, 9), range(3, 8)),
                        ]
                    else:
                        c0_passes = [(range(9), range(N_PIX_TILES))]
                    for ci in range(2):
                        passes = (
                            c0_passes if ci == 0 else [(range(9), range(N_PIX_TILES))]
                        )
                        for pos_range, t_range in passes:
                            for pos in pos_range:
                                lhsT = wl[ci][:, pos, ms:me]
                                for t in t_range:
                                    mm(lhsT, xp[ci], pos // 3, pos % 3, t)
                    tail_mms(range(N_PIX_TILES))
                    assert all(k == n_acc for k in acc_k)
                    for t in range(N_PIX_TILES):
                        epilogue(t)
                else:
                    # last co chunk tile-by-tile: each PSUM tile finishes its
                    # 23 accumulations early so the Identity+bias epilogue
                    # overlaps the remaining stream instead of trailing it
                    for t in range(N_PIX_TILES):
                        for ci in range(2):
                            for pos in range(9):
                                mm(wl[ci][:, pos, ms:me], xp[ci], pos // 3, pos % 3, t)
                        tail_mms([t])
                        epilogue(t)
                    assert all(k == n_acc for k in acc_k)

    nc.compile()
    return nc


_NC_CACHE = None


def _get_nc():
    global _NC_CACHE
    if _NC_CACHE is None:
        _NC_CACHE = build()
    return _NC_CACHE


def _prep_in_maps(x, w_bits, b_bits):
    # w_bits [co, ci, kh, kw] -> [ci, kh*3+kw, co] u8 codes (host relayout only)
    w9 = np.ascontiguousarray(
        w_bits.astype(np.uint8).transpose(1, 2, 3, 0).reshape(CIN, 9, COUT)
    )
    b2 = np.zeros((3 * 128, 1), np.uint8)
    b2[:COUT, 0] = b_bits.astype(np.uint8).reshape(COUT)
    return [
        {
            "x": np.ascontiguousarray(x[i].reshape(CIN, PIX).astype(np.float32)),
            "w9": w9,
            "b": b2,
        }
        for i in range(B)
    ]


def kernel(x, w_bits, b_bits):
    nc = _get_nc()
    in_maps = _prep_in_maps(x, w_bits, b_bits)
    res = run_bass_kernel_spmd(nc, in_maps, core_ids=list(range(B)), trace=False)
    return np.stack(
        [res.results[i]["out"].reshape(COUT, H, W) for i in range(B)]
    ).astype(np.float32)


if __name__ == "__main__":
    rng = np.random.default_rng(0)
    x = rng.standard_normal((B, CIN, H, W)).astype(np.float32)
    w_bits = rng.integers(0, 256, (COUT, CIN, 3, 3)).astype(np.int32)
    b_bits = rng.integers(0, 256, (COUT,)).astype(np.int32)
    out = kernel(x, w_bits, b_bits)
    print("out", out.shape, out.dtype, float(np.abs(out).mean()))



# revision 14
# speedup vs baseline: 1.0159x; 1.0028x over previous
"""Trainium2 Bass kernel for nn_Conv2d_14147622273082.

Conv2d 3x3, stride 1, pad 1: x [8, 320, 64, 64] f32, hf8-coded weights
w_bits [320, 320, 3, 3] i32 (codes 0..255), bias codes b_bits [320] i32.
out = conv2d(x, hf8_decode(w_bits)) + hf8_decode(b_bits).

Strategy: data-parallel over batch (1 image per NeuronCore, 8 cores).
Weights replicated; hf8 decode on-device via a bit trick:
hf8(1-4-3, bias 14) == bitcast_f32(sign<<31 | code7<<20) * 2^113
(exact, incl. subnormals). The conv is 9 shifted [Cin,Cout] x [Cin,pix]
fp16 matmuls accumulated in PSUM over a zero-padded fp16 input image.

Cin=320 splits into K-chunks (128, 128, 64). The 64-wide tail would waste
half the PE array, so kernel positions are packed in pairs: partitions
0:64 hold the tail channels, partitions 64:128 hold the same channels
with the padded image pre-shifted by the delta between the two positions
(flat +1 == next column; flat +66 == (row+1, col-2) in the 68-wide pad),
so one K=128 matmul computes two positions at once. 9 positions -> 4
pairs + 1 solo: 23 instead of 27 accumulating matmuls per PSUM tile.
"""

import numpy as np

import concourse.bass as bass
import concourse.tile as tile
from concourse import bacc, mybir
from concourse.bass_utils import run_bass_kernel_spmd

B, CIN, COUT, H, W = 8, 320, 320, 64, 64
PIX = H * W  # 4096
P = 128
CO_CHUNKS = [(0, 128), (128, 256), (256, 320)]
N_TILE = 512  # pixels per psum tile = 8 rows of 64
ROWS_PER_TILE = N_TILE // W  # 8
N_PIX_TILES = PIX // N_TILE  # 8
# padded image: rows 0..65 (top/bottom zero), cols: 2 left / 2 right zero
HP, WP = H + 2, W + 4  # 66 x 68 (even left pad keeps fp16 writes 4B-aligned)

# tail position pairing: (pos_a, pos_b) packed on partitions (0:64, 64:128).
# delta = flat_offset(b) - flat_offset(a) in the padded [66,68] layout.
# pairs with delta 1 share the "+1 shifted" upper image (xp2 upper half);
# the delta-66 pair gets its own tile (XB).
TAIL_PAIRS = [(0, 1), (2, 3), (4, 5), (6, 7)]  # pos = kh*3+kw
TAIL_SOLO = 8

F16 = mybir.dt.float16
F32 = mybir.dt.float32
I32 = mybir.dt.int32
U8 = mybir.dt.uint8
HF8_SCALE = float(2.0**113)
N_WARMUP = 70


def _decode_hf8(nc, pool, codes_ap, out_ap, nparts, free, tag, after=None):
    """out = hf8_decode(codes) = bitcast_f32(sign<<31 | code7<<20) * 2^113.

    Returns the last instruction. If `after` is given, the stage's first ops
    get no-sync ordering edges onto it so the Tile scheduler cannot hoist
    this stage ahead of earlier work on the engine (its compile-time DMA
    timing model underestimates HBM contention, which otherwise causes
    head-of-line stalls).
    """
    from concourse.tile_rust import add_dep_helper

    t0 = pool.tile([P, free], I32, tag=f"{tag}_t0", name=f"{tag}_t0")
    t1 = pool.tile([P, free], I32, tag=f"{tag}_t1", name=f"{tag}_t1")
    t2 = pool.tile([P, free], I32, tag=f"{tag}_t2", name=f"{tag}_t2")
    # widen u8 codes -> i32 (DVE converts via f32; 0..255 exact)
    i0 = nc.vector.tensor_copy(t0[:nparts], codes_ap)
    i1 = nc.vector.tensor_scalar(
        t1[:nparts], t0[:nparts], 0x80, 24,
        mybir.AluOpType.bitwise_and, mybir.AluOpType.logical_shift_left,
    )
    i2 = nc.vector.tensor_scalar(
        t2[:nparts], t0[:nparts], 0x7F, 20,
        mybir.AluOpType.bitwise_and, mybir.AluOpType.logical_shift_left,
    )
    if after is not None:
        add_dep_helper(i0.ins, after.ins, sync=False, reason="decode stage order")
    nc.vector.tensor_tensor(
        t1[:nparts], t1[:nparts], t2[:nparts], mybir.AluOpType.bitwise_or
    )
    return nc.vector.tensor_scalar_mul(out_ap, t1[:nparts].bitcast(F32), HF8_SCALE)


def _pad_borders(nc, xt, col_lo, col_hi, parts=slice(0, P), rows=(0, HP - 1)):
    """Zero the pad borders around an interior written at cols [col_lo, col_hi).

    Runs on GpSimd: the DVE prologue queue is the critical path to the first
    weight decode, and these ~20 memsets were delaying it by ~4us.
    """
    nc.gpsimd.memset(xt[parts, rows[0] : rows[0] + 1, :], 0.0)
    nc.gpsimd.memset(xt[parts, rows[1] : rows[1] + 1, :], 0.0)
    if col_lo > 0:
        nc.gpsimd.memset(xt[parts, rows[0] + 1 : rows[1], 0:col_lo], 0.0)
    if col_hi < WP:
        nc.gpsimd.memset(xt[parts, rows[0] + 1 : rows[1], col_hi:WP], 0.0)


def build():
    from concourse.tile_rust import add_dep_helper

    nc = bacc.Bacc(
        "TRN2", target_bir_lowering=False, debug=False, enable_partition_id=False
    )
    x_d = nc.dram_tensor("x", [CIN, PIX], F32, kind="ExternalInput")
    w_d = nc.dram_tensor("w9", [CIN, 9, COUT], U8, kind="ExternalInput")
    b_d = nc.dram_tensor("b", [3 * P, 1], U8, kind="ExternalInput")
    out_d = nc.dram_tensor("out", [COUT, PIX], F32, kind="ExternalOutput")

    with tile.TileContext(nc) as tc:
        with (
            tc.tile_pool(name="persist", bufs=1) as persist,
            tc.tile_pool(name="stage", bufs=1) as stage,
            tc.tile_pool(name="outsb", bufs=4) as outsb,
            tc.tile_pool(name="psum", bufs=1, space="PSUM") as psum_pool,
        ):
            # All input DMAs ride the sync queue, which processes them in
            # issue order: earliest-deadline first. hf8 decode runs on DVE,
            # pad casts on the Scalar engine, PSUM epilogue on Scalar.
            # ---- SBUF tiles ----
            wraw = [
                stage.tile([P, 9, COUT], U8, tag=f"wraw{c}", name=f"wraw{c}")
                for c in range(2)
            ]
            wt = [
                persist.tile([P, 9, COUT], F16, tag=f"wl{c}", name=f"wl{c}")
                for c in range(2)
            ]
            xs = [
                stage.tile([P, H, W], F32, tag=f"xstage{c}", name=f"xstage{c}")
                for c in range(2)
            ]
            xt = [
                persist.tile([P, HP, WP], F16, tag=f"xpad{c}", name=f"xpad{c}")
                for c in range(2)
            ]
            wraw2 = stage.tile([P, 5, COUT], U8, tag="wraw2", name="wraw2")
            wpair = persist.tile([P, 5, COUT], F16, tag="wpair", name="wpair")
            xs2 = stage.tile([P, H, W], F32, tag="xstage2", name="xstage2")
            xp2 = persist.tile([P, HP, WP], F16, tag="xpad2", name="xpad2")
            xb2 = persist.tile([P, HP, WP], F16, tag="xpadb", name="xpadb")
            wl = wt
            xp = xt

            # ---- input DMAs, deadline order, one in-order queue ----
            # First units are small so the first matmuls (pos 0-2 x tiles 0-1)
            # can start ~11us in: w pos 0-2, then x rows 0-16.
            wfl = [w.rearrange("p a b -> p (a b)") for w in wraw]
            q = 3 * COUT
            R1 = 17  # first x slab rows (tiles 0-1 need src rows <= 16)
            nc.sync.dma_start(wfl[0][:, :q], w_d[0:P, :3])
            nc.sync.dma_start(
                xs[0][:, :R1],
                x_d[0:P, : R1 * W].rearrange("p (h w) -> p h w", h=R1),
            )
            nc.sync.dma_start(wfl[0][:, q:], w_d[0:P, 3:])
            nc.sync.dma_start(
                xs[0][:, R1 : H // 2],
                x_d[0:P, R1 * W : PIX // 2].rearrange(
                    "p (h w) -> p h w", h=H // 2 - R1
                ),
            )
            nc.sync.dma_start(
                xs[0][:, H // 2 :],
                x_d[0:P, PIX // 2 :].rearrange("p (h w) -> p h w", h=H // 2),
            )
            nc.sync.dma_start(wraw[1][:], w_d[P : 2 * P])
            nc.sync.dma_start(
                xs[1][:], x_d[P : 2 * P].rearrange("p (h w) -> p h w", h=H)
            )
            cs, ce = 256, 320
            nc.sync.dma_start(wraw2[0:64, 0:5], w_d[cs:ce, 0:9:2])
            nc.sync.dma_start(wraw2[64:128, 0:4], w_d[cs:ce, 1:9:2])
            nc.sync.dma_start(
                xs2[0:64], x_d[cs:ce].rearrange("p (h w) -> p h w", h=H)
            )
            nc.sync.dma_start(
                xs2[64:128], x_d[cs:ce].rearrange("p (h w) -> p h w", h=H)
            )
            braw = stage.tile([P, 3], U8, tag="braw", name="braw")
            nc.sync.dma_start(
                braw[:], b_d.rearrange("(a p) one -> p (a one)", p=P)
            )

            # ---- PE warmup: keep TensorE busy (HAM at 8/8) through the
            # prologue so the real stream starts at 2.4 GHz ----
            wsrc = stage.tile([P, P], F16, tag="wsrc", name="wsrc")
            nc.gpsimd.memset(wsrc[:], 0.0)
            warm_ps = psum_pool.tile([P, N_TILE], F32, tag="acc0", name="warm_ps")
            for _ in range(N_WARMUP):
                nc.tensor.matmul(
                    warm_ps[:, 0:P], wsrc[:], wsrc[:], start=True, stop=True
                )

            # ---- borders (GpSimd, no data deps: fills the DMA wait) ----
            for c in range(2):
                _pad_borders(nc, xt[c], 2, W + 2)
            _pad_borders(nc, xp2, 2, W + 2, parts=slice(0, 64))
            _pad_borders(nc, xp2, 1, W + 1, parts=slice(64, P))
            _pad_borders(nc, xb2, 2, W + 2, parts=slice(0, 64))
            nc.gpsimd.memset(xb2[64:128, H : HP, :], 0.0)
            nc.gpsimd.memset(xb2[64:128, 0:H, 0:4], 0.0)
            nc.gpsimd.memset(wraw2[64:128, 4], 0)

            # ---- Scalar-engine casts (warm the Copy table first), chained
            # in deadline order so the static schedule matches reality ----
            warm = stage.tile([P, 1], F32, tag="warm", name="warm")
            nc.gpsimd.memset(warm[:], 0.0)
            a0 = nc.scalar.copy(warm[:], warm[:])
            a1 = nc.scalar.copy(
                xt[0][:, 1 : R1 + 1, 2 : W + 2], xs[0][:, :R1]
            )
            a1b = nc.scalar.copy(
                xt[0][:, R1 + 1 : H // 2 + 1, 2 : W + 2], xs[0][:, R1 : H // 2]
            )
            a2 = nc.scalar.copy(
                xt[0][:, H // 2 + 1 : H + 1, 2 : W + 2], xs[0][:, H // 2 :]
            )
            a3 = nc.scalar.copy(xt[1][:, 1 : H + 1, 2 : W + 2], xs[1][:])
            a4 = nc.scalar.copy(xp2[0:64, 1 : H + 1, 2 : W + 2], xs2[0:64])
            a5 = nc.scalar.copy(xp2[64:128, 1 : H + 1, 1 : W + 1], xs2[64:128])
            a6 = nc.scalar.copy(xb2[64:128, 0:H, 4:WP], xs2[64:128])
            prev = a0
            for a in (a1, a1b, a2, a3, a4, a5, a6):
                add_dep_helper(a.ins, prev.ins, sync=False, reason="cast order")
                prev = a

            # ---- hf8 decode on DVE, stage-chained in deadline order ----
            d1 = _decode_hf8(
                nc, stage, wfl[0][:, :q],
                wt[0].rearrange("p a b -> p (a b)")[:, :q], P, q, "wdec",
            )
            d2 = _decode_hf8(
                nc, stage, wfl[0][:, q:],
                wt[0].rearrange("p a b -> p (a b)")[:, q:],
                P, 9 * COUT - q, "wdec", after=d1,
            )
            d3 = _decode_hf8(
                nc, stage, wfl[1],
                wt[1].rearrange("p a b -> p (a b)"), P, 9 * COUT, "wdec", after=d2,
            )
            d4 = _decode_hf8(
                nc, stage,
                wraw2.rearrange("p a b -> p (a b)"),
                wpair.rearrange("p a b -> p (a b)"), P, 5 * COUT, "wdec2", after=d3,
            )
            # xb2 lower = same padded image as xp2 lower (same partitions)
            cpy = nc.vector.tensor_copy(
                xb2[0:64, 1 : H + 1, 2 : W + 2], xp2[0:64, 1 : H + 1, 2 : W + 2]
            )
            add_dep_helper(cpy.ins, d4.ins, sync=False, reason="tail copy order")
            bias = []
            prev = None
            for mi, (ms, me) in enumerate(CO_CHUNKS):
                pm = me - ms
                bf = persist.tile([P, 1], F32, tag=f"bias{mi}", name=f"bias{mi}")
                prev = _decode_hf8(
                    nc, stage, braw[:pm, mi : mi + 1], bf[:pm], pm, 1, "bdec",
                    after=prev if prev is not None else d4,
                )
                bias.append(bf)

            # ---- matmuls: out[co, pix] += w[ci,co].T @ x_shift[ci, pix] ----
            n_acc = 2 * 9 + len(TAIL_PAIRS) + 1  # 23 per psum tile
            for mi, (ms, me) in enumerate(CO_CHUNKS):
                pm = me - ms
                acc = [
                    psum_pool.tile(
                        [P, N_TILE], F32, tag=f"acc{t}", name=f"acc_{mi}_{t}"
                    )
                    for t in range(N_PIX_TILES)
                ]
                acc_k = [0] * N_PIX_TILES

                def mm(lhsT, src, kh, kw, t, pm=pm, acc=acc, acc_k=acc_k):
                    h0 = t * ROWS_PER_TILE
                    rhs = src[
                        : lhsT.shape[0],
                        h0 + kh : h0 + kh + ROWS_PER_TILE,
                        kw + 1 : kw + 1 + W,
                    ]
                    nc.tensor.matmul(
                        acc[t][:pm], lhsT, rhs,
                        start=(acc_k[t] == 0), stop=(acc_k[t] == n_acc - 1),
                    )
                    acc_k[t] += 1

                # For the very first co chunk, order chunk-0 work as
                # (weight half x image half) passes: the first 20 matmuls
                # need only the first 5 decoded positions and the first half
                # of the chunk-0 image.
                def tail_mms(t_range, pm=pm):
                    for j, (pa, pb) in enumerate(TAIL_PAIRS):
                        kh, kw = pa // 3, pa % 3
                        src = xb2 if (pa, pb) == (2, 3) else xp2
                        for t in t_range:
                            mm(wpair[:, j, ms:me], src, kh, kw, t)
                    for t in t_range:
                        mm(wpair[0:64, 4, ms:me], xp2, 2, 2, t)

                def epilogue(t, pm=pm, ms=ms, mi=mi):
                    osb = outsb.tile([P, N_TILE], F32, tag="osb", name="osb")
                    nc.scalar.activation(
                        osb[:pm], acc[t][:pm],
                        mybir.ActivationFunctionType.Identity,
                        bias=bias[mi][:pm], scale=1.0,
                    )
                    nc.sync.dma_start(
                        out_d[ms : ms + pm, t * N_TILE : (t + 1) * N_TILE], osb[:pm]
                    )

                if mi < 2:
                    if mi == 0:
                        # staged so the first matmuls need only w pos 0-2 and
                        # x rows 0-16 (tiles 0-1), matching the DMA/decode/
                        # cast splits above
                        c0_passes = [
                            (range(0, 3), range(0, 2)),
                            (range(3, 9), range(0, 2)),
                            (range(0, 9), range(2, 4)),
                            (range(0, 9), range(4, 8)),
                        ]
                    else:
                        c0_passes = [(range(9), range(N_PIX_TILES))]
                    for ci in range(2):
                        passes = (
                            c0_passes if ci == 0 else [(range(9), range(N_PIX_TILES))]
                        )
                        for pos_range, t_range in passes:
                            for pos in pos_range:
                                lhsT = wl[ci][:, pos, ms:me]
                                for t in t_range:
                                    mm(lhsT, xp[ci], pos // 3, pos % 3, t)
                    tail_mms(range(N_PIX_TILES))
                    assert all(k == n_acc for k in acc_k)
                    for t in range(N_PIX_TILES):
                        epilogue(t)
                else:
                    # last co chunk tile-by-tile: each PSUM tile finishes its
                    # 23 accumulations early so the Identity+bias epilogue
                    # overlaps the remaining stream instead of trailing it
                    for t in range(N_PIX_TILES):
                        for ci in range(2):
                            for pos in range(9):
                                mm(wl[ci][:, pos, ms:me], xp[ci], pos // 3, pos % 3, t)
                        tail_mms([t])
                        epilogue(t)
                    assert all(k == n_acc for k in acc_k)

    nc.compile()
    return nc


_NC_CACHE = None


def _get_nc():
    global _NC_CACHE
    if _NC_CACHE is None:
        _NC_CACHE = build()
    return _NC_CACHE


def _prep_in_maps(x, w_bits, b_bits):
    # w_bits [co, ci, kh, kw] -> [ci, kh*3+kw, co] u8 codes (host relayout only)
    w9 = np.ascontiguousarray(
        w_bits.astype(np.uint8).transpose(1, 2, 3, 0).reshape(CIN, 9, COUT)
    )
    b2 = np.zeros((3 * 128, 1), np.uint8)
    b2[:COUT, 0] = b_bits.astype(np.uint8).reshape(COUT)
    return [
        {
            "x": np.ascontiguousarray(x[i].reshape(CIN, PIX).astype(np.float32)),
            "w9": w9,
            "b": b2,
        }
        for i in range(B)
    ]


def kernel(x, w_bits, b_bits):
    nc = _get_nc()
    in_maps = _prep_in_maps(x, w_bits, b_bits)
    res = run_bass_kernel_spmd(nc, in_maps, core_ids=list(range(B)), trace=False)
    return np.stack(
        [res.results[i]["out"].reshape(COUT, H, W) for i in range(B)]
    ).astype(np.float32)


if __name__ == "__main__":
    rng = np.random.default_rng(0)
    x = rng.standard_normal((B, CIN, H, W)).astype(np.float32)
    w_bits = rng.integers(0, 256, (COUT, CIN, 3, 3)).astype(np.int32)
    b_bits = rng.integers(0, 256, (COUT,)).astype(np.int32)
    out = kernel(x, w_bits, b_bits)
    print("out", out.shape, out.dtype, float(np.abs(out).mean()))

